# revision 23
# baseline (speedup 1.0000x reference)
"""Causal self-attention on 8 TRN2 NeuronCores.

Sharding: B=4 batches x 16 heads -> 64 (b,h) pairs; core c handles batch
b=c//2 and head-group hg=c%2 (8 heads = 512 of the 1024 features).
Q/K/V projection weights are row-sliced per head group (column-sharded in
the x @ W.T sense), so each core computes its own (b, 8-head) slice of the
S x S attention with no cross-core communication.

Kernel design:
- Matmuls contract over SBUF partitions, so X and the weight slices are
  shipped pre-transposed ([H, S] / [H, F]) in bf16; X^T / W^T tiles then
  load with plain (fast) DMA. Projections run bf16 x bf16 with fp32 PSUM
  accumulation; Q^T/K^T are rounded to bf16 (QK gets the FWL fast weight
  load), biases are added in fp32 during the PSUM->SBUF copyback.
- Scores are computed transposed, S^T[k, q] = (K^T)^T Q^T per 128-key
  chunk with a 512-wide q window; head parity picks partitions 0-63 vs
  64-127, whose K=64 matmuls run concurrently in separate PE row groups.
- No row-max subtraction: scaled scores are ~N(0,1), exp is safe in fp32.
  exp runs on ScalarE straight from PSUM with the attention-mask bias and
  the 1/sqrt(64) scale fused in. On diagonal tiles the q window of
  QK/exp/AV is narrowed; only one partial 128-col window per head needs a
  0/1 causal-mask multiply after exp.
- AV runs in natural layout: out[q, d+1] += (P^T chunk).T @ V_aug with a
  ones-column appended to V, so each accumulator's column HD is the
  softmax denominator; normalize = reciprocal + per-partition scalar mul,
  no output transposes. Full 128-row PE utilization.
- The kernel is ScalarE-bound: the exp stream (~160us of ACT) is the
  critical path. All other work (projections of later pairs, V chunks,
  AV/normalize/output) is drained from a fill queue BETWEEN QK steps,
  paced by estimated PE cost (~a QK period's worth per step) so the exp
  cadence stays tight, with explicit deadline markers forcing units
  needed by an upcoming QK step to drain first. This keeps TensorE dense
  (HAM clock stays at 2.4 GHz) without letting fill bursts starve exp.
"""

import sys

if "/opt/trn_rl_repo" not in sys.path:
    sys.path.insert(0, "/opt/trn_rl_repo")

import numpy as np
import ml_dtypes

_bf16 = np.dtype(ml_dtypes.bfloat16)

B, S, H, NH = 4, 2048, 1024, 16
HD = 64
NCORES = 8
F = 512  # features per core (8 heads)
NHEADS = 8  # heads per core
NPAIR = 4  # head pairs per core
HCH = H // 128  # 8 hidden chunks
SCH = S // 128  # 16 sequence chunks
P = 128

_CACHE = {}

# fill pacing: estimated PE-ns drained between consecutive QK steps.
# DQ_QUOTA bounds how much deadline (projection) work runs per step; the
# rest of TOTAL goes to elastic work (V chunks, AV/normalize/output).
# Keeping QK+fill under the ~1.1us exp period keeps the next QK punctual
# so the exp stream never data-waits; leftovers drain in the last pair's
# window (bigger budget there) while exp still covers them.
DQ_QUOTA = 450
TOTAL_BUDGET = 760
LAST_BUDGET = 900


def _build_bass():
    import concourse.tile as tile
    from concourse import bacc, mybir
    from contextlib import ExitStack
    from collections import deque

    f32 = mybir.dt.float32
    EXP = mybir.ActivationFunctionType.Exp
    ADD = mybir.AluOpType.add
    MULT = mybir.AluOpType.mult

    nc = bacc.Bacc("TRN2", target_bir_lowering=False, debug=False, num_devices=NCORES)

    bf16 = mybir.dt.bfloat16
    x_d = nc.dram_tensor("xtb", [H, S], bf16, kind="ExternalInput").ap()
    wq_d = nc.dram_tensor("wqtb", [H, F], bf16, kind="ExternalInput").ap()
    wk_d = nc.dram_tensor("wktb", [H, F], bf16, kind="ExternalInput").ap()
    wv_d = nc.dram_tensor("wvtb", [H, F], bf16, kind="ExternalInput").ap()
    bqt_d = nc.dram_tensor("bqt", [P, NPAIR], f32, kind="ExternalInput").ap()
    bkt_d = nc.dram_tensor("bkt", [P, NPAIR], f32, kind="ExternalInput").ap()
    bvb_d = nc.dram_tensor("bvb", [P, F], f32, kind="ExternalInput").ap()
    maskb_d = nc.dram_tensor("maskb", [P, SCH], f32, kind="ExternalInput").ap()
    cm_d = nc.dram_tensor("cm", [P, P], bf16, kind="ExternalInput").ap()
    out_d = nc.dram_tensor("out", [S, F], f32, kind="ExternalOutput").ap()

    with tile.TileContext(nc) as tc, ExitStack() as ctx:
        const = ctx.enter_context(tc.tile_pool(name="const", bufs=1))
        xt_pool = ctx.enter_context(tc.tile_pool(name="xt", bufs=1))
        xt = xt_pool.tile([P, HCH, S], bf16, tag="xt")  # X^T via DMA transpose
        v_pool = ctx.enter_context(tc.tile_pool(name="v", bufs=1))
        v = v_pool.tile([P, SCH, NHEADS, HD + 1], bf16, tag="v")  # V + ones col

        # PSUM: QK pair slots (2 x 2 banks), small slots for projections and
        # V chunks (2 x 1 bank), AV accumulators (2 x 1 bank) = 8 banks.
        mmps = ctx.enter_context(tc.tile_pool(name="mmps", bufs=2, space="PSUM"))
        smps = ctx.enter_context(tc.tile_pool(name="smps", bufs=2, space="PSUM"))
        ops_ = ctx.enter_context(tc.tile_pool(name="ops", bufs=2, space="PSUM"))
        wt_pool = ctx.enter_context(tc.tile_pool(name="wt", bufs=3))
        qkt_pool = ctx.enter_context(tc.tile_pool(name="qkt", bufs=3))
        p_pool = ctx.enter_context(tc.tile_pool(name="pp", bufs=36))
        wtv_pool = ctx.enter_context(tc.tile_pool(name="wtv", bufs=1))
        obuf = ctx.enter_context(tc.tile_pool(name="obuf", bufs=8))
        rec_pool = ctx.enter_context(tc.tile_pool(name="rec", bufs=8))

        # ---- critical-path DMAs first: pair-0 weights + X^T block 0 (split
        # per hidden chunk so the first projection matmuls start after
        # ~160KB instead of 1.5MB), then the small consts. ----
        wtq0 = wt_pool.tile([P, HCH, P], bf16, tag="wtq")
        wtk0 = wt_pool.tile([P, HCH, P], bf16, tag="wtk")
        nc.sync.dma_start(wtq0[:], wq_d[:, 0:128].rearrange("(c p) f -> p c f", p=P))
        nc.sync.dma_start(wtk0[:], wk_d[:, 0:128].rearrange("(c p) f -> p c f", p=P))
        for j in range(HCH):
            nc.sync.dma_start(
                xt[:, j, 0:512],
                x_d[j * 128 : (j + 1) * 128, 0:512],
            )
        bqt = const.tile([P, NPAIR], f32, tag="bqt")
        nc.sync.dma_start(bqt[:], bqt_d[:])
        bkt = const.tile([P, NPAIR], f32, tag="bkt")
        nc.sync.dma_start(bkt[:], bkt_d[:])
        maskb = const.tile([P, SCH], f32, tag="maskb")
        nc.sync.dma_start(maskb[:], maskb_d[:])
        cm = const.tile([P, P], bf16, tag="cm")
        nc.sync.dma_start(cm[:], cm_d[:])
        for j in range(HCH):
            nc.sync.dma_start(
                xt[:, j, 512:1024],
                x_d[j * 128 : (j + 1) * 128, 512:1024],
            )
        bvb = const.tile([P, F], f32, tag="bvb")
        nc.sync.dma_start(bvb[:], bvb_d[:])
        wtv = wtv_pool.tile([P, HCH, F], bf16, tag="wtv")
        nc.sync.dma_start(wtv[:], wv_d.rearrange("(c p) f -> p c f", p=P))
        for sb in (2, 3):
            nc.sync.dma_start(
                xt[:, :, sb * 512 : (sb + 1) * 512],
                x_d[:, sb * 512 : (sb + 1) * 512].rearrange("(c p) s -> p c s", p=P),
            )

        # V ones-column (softmax denominator trick): no input dependency
        nc.vector.memset(v[:, :, :, HD : HD + 1], 1.0)

        # ---- two fill queues: dq carries deadline work (W DMAs + Q/K
        # projections, tracked by markers so units land before the QK step
        # that reads them); eq carries elastic work (V chunks, AV units)
        # that may spill across pair windows without stalling exp. ----
        dq = deque()
        eq = deque()
        state = {"denq": 0, "dpop": 0}
        need = {}  # (pr, 'q'|'k', st) -> dq count required

        def push_d(cost, fn, marker=None):
            dq.append((cost, fn))
            state["denq"] += 1
            if marker is not None:
                need[marker] = state["denq"]

        def push_e(cost, fn):
            eq.append((cost, fn))

        def drain_to(n):
            while state["dpop"] < n and dq:
                _, fn = dq.popleft()
                state["dpop"] += 1
                fn()

        def drain_dq_ns(budget):
            acc = 0
            while dq and acc < budget:
                cost, fn = dq.popleft()
                state["dpop"] += 1
                fn()
                acc += cost
            return acc

        def drain_eq_ns(budget):
            acc = 0
            while eq and acc < budget:
                cost, fn = eq.popleft()
                fn()
                acc += cost

        def need_for(pr, qi, kc):
            r = 0
            m = need.get((pr, "q", qi))
            if m is not None:
                r = max(r, m)
            m = need.get((pr, "k", kc // 4))
            if m is not None:
                r = max(r, m)
            return r

        # ---- per head-pair: project Q^T/K^T; units are (cost, fn, marker)
        def make_pair_proj(pr, wt_tiles=None):
            if wt_tiles is None:
                wtq = wt_pool.tile([P, HCH, P], bf16, tag="wtq")
                wtk = wt_pool.tile([P, HCH, P], bf16, tag="wtk")
            else:
                wtq, wtk = wt_tiles
            qt = qkt_pool.tile([P, S], bf16, tag="qt")
            kt = qkt_pool.tile([P, S], bf16, tag="kt")
            units = []
            if wt_tiles is None:
                for wd, wt in ((wq_d, wtq), (wk_d, wtk)):

                    def dma_u(wt=wt, wd=wd):
                        nc.sync.dma_start(
                            wt[:],
                            wd[:, pr * 128 : (pr + 1) * 128].rearrange(
                                "(c p) f -> p c f", p=P
                            ),
                        )

                    units.append((100, dma_u, None))
            for st in range(4):
                for wt, dst, bias, tag in (
                    (wtq, qt, bqt, "q"),
                    (wtk, kt, bkt, "k"),
                ):
                    ps = smps.tile([P, F], f32, tag="sm")
                    for j0 in range(0, HCH, 2):

                        def mm_u(wt=wt, ps=ps, st=st, j0=j0):
                            for j in (j0, j0 + 1):
                                nc.tensor.matmul(
                                    ps[:],
                                    wt[:, j, :],
                                    xt[:, j, st * 512 : (st + 1) * 512],
                                    start=(j == 0),
                                    stop=(j == HCH - 1),
                                )

                        units.append((440, mm_u, None))

                    def cb_u(dst=dst, ps=ps, st=st, bias=bias):
                        nc.vector.tensor_scalar_add(
                            dst[:, st * 512 : (st + 1) * 512],
                            ps[:],
                            bias[:, pr : pr + 1],
                        )

                    units.append((80, cb_u, (pr, tag, st)))
            return qt, kt, units

        def v_unit(si, ha=0, hn=NHEADS):
            def u():
                w = hn * HD
                ps = smps.tile([P, F], f32, tag="sm")
                for j in range(HCH):
                    nc.tensor.matmul(
                        ps[:, 0:w],
                        xt[:, j, si * 128 : (si + 1) * 128],
                        wtv[:, j, ha * HD : ha * HD + w],
                        start=(j == 0),
                        stop=(j == HCH - 1),
                    )
                nc.vector.tensor_tensor(
                    v[:, si, ha : ha + hn, 0:HD],
                    ps[:, 0:w].rearrange("p (h d) -> p h d", h=hn),
                    bvb[:, ha * HD : ha * HD + w].rearrange("p (h d) -> p h d", h=hn),
                    ADD,
                )

            return u

        # ---- A0: pair-0 st0 projection runs inline (gates the first QK);
        # st1-st3 go into the deadline queue, the V chunks for heads 0-3
        # into the elastic queue (heads 4-5 / 6-7 are pushed at the pair-1
        # / pair-2 window starts — they are only read by later pairs' AV).
        pair_state = {0: make_pair_proj(0, wt_tiles=(wtq0, wtk0))}
        p0u = pair_state[0][2]
        assert len(p0u) == 40
        for _, fn, _m in p0u[0:10]:  # st0 inline
            fn()
        for c, fn, m in p0u[10:40]:  # st1-st3
            push_d(c, fn, m)
        for si in range(SCH):
            push_e(900, v_unit(si, 0, 4))

        def make_av_unit(pts, qc, h, hb, q0, qi):
            def av_unit():
                nkq = 4 * qi + qc + 1
                o_ps = ops_.tile([P, HD + 1], f32, tag="o")
                for kc in range(nkq):
                    nc.tensor.matmul(
                        o_ps[:],
                        pts[kc][:, hb + qc * 128 : hb + (qc + 1) * 128],
                        v[:, kc, h, :],
                        start=(kc == 0),
                        stop=(kc == nkq - 1),
                    )
                rec = rec_pool.tile([P, 1], f32, tag="rec")
                nc.vector.reciprocal(rec[:], o_ps[:, HD : HD + 1])
                otile = obuf.tile([P, HD], f32, tag="ob")
                nc.vector.tensor_scalar_mul(otile[:], o_ps[:, 0:HD], rec[:])
                nc.sync.dma_start(
                    out_d[q0 + qc * 128 : q0 + (qc + 1) * 128, h * HD : (h + 1) * HD],
                    otile[:],
                )

            return av_unit

        for pr in range(NPAIR):
            qt, kt = pair_state[pr][0], pair_state[pr][1]
            if pr > 0:
                for c, fn, m in pair_state[pr][2][22:42]:  # own st2/st3
                    push_d(c, fn, m)
            if pr == 1:
                for si in range(SCH):
                    push_e(500, v_unit(si, 4, 2))
            elif pr == 2:
                for si in range(SCH):
                    push_e(500, v_unit(si, 6, 2))
            if pr + 1 < NPAIR:
                pair_state[pr + 1] = make_pair_proj(pr + 1)
                for c, fn, m in pair_state[pr + 1][2][:22]:  # W DMA + st0/st1
                    push_d(c, fn, m)
            h0, h1 = 2 * pr, 2 * pr + 1

            def emit_qk(qi, kc, qt=qt, kt=kt):
                q0 = qi * 512
                off = kc - 4 * qi
                lo = off * 128 if off > 0 else 0
                ps = mmps.tile([P, 1024], f32, tag="mm")
                nc.tensor.matmul(
                    ps[:, lo:512],
                    kt[0:64, kc * 128 : (kc + 1) * 128],
                    qt[0:64, q0 + lo : q0 + 512],
                    start=True,
                    stop=True,
                    skip_group_check=True,
                )
                nc.tensor.matmul(
                    ps[:, 512 + lo : 1024],
                    kt[64:128, kc * 128 : (kc + 1) * 128],
                    qt[64:128, q0 + lo : q0 + 512],
                    start=True,
                    stop=True,
                    skip_group_check=True,
                )
                return ps

            steps = [(qi, kc) for qi in range(4) for kc in range(4 * (qi + 1))]
            pts_by_qi = {qi: [] for qi in range(4)}
            drain_to(need_for(pr, *steps[0]))
            ps = emit_qk(*steps[0])
            for i, (qi, kc) in enumerate(steps):
                q0 = qi * 512
                off = kc - 4 * qi
                lo = off * 128 if off > 0 else 0
                pt = p_pool.tile([P, 1024], bf16, tag="pt")
                pts_by_qi[qi].append(pt)
                if lo == 0:
                    nc.scalar.activation(
                        pt[:], ps[:], EXP, bias=maskb[:, kc : kc + 1], scale=0.125
                    )
                else:
                    nc.scalar.activation(
                        pt[:].rearrange("p (t q) -> p t q", t=2)[:, :, lo:512],
                        ps[:].rearrange("p (t q) -> p t q", t=2)[:, :, lo:512],
                        EXP,
                        bias=maskb[:, kc : kc + 1],
                        scale=0.125,
                    )
                if off >= 0:
                    pv = pt[:].rearrange("p (t q) -> p t q", t=2)[:, :, lo : lo + 128]
                    nc.vector.tensor_mul(
                        pv, pv, cm[:, None, :].to_broadcast((P, 2, P))
                    )
                    # all keys for q-chunk `off` of this q-tile are now in
                    # flight -> its AV units can be scheduled
                    nkq = 4 * qi + off + 1
                    for h, hb in ((h0, 0), (h1, 512)):
                        push_e(
                            35 * nkq + 250,
                            make_av_unit(pts_by_qi[qi], off, h, hb, q0, qi),
                        )
                if i + 1 < len(steps):
                    drain_to(need_for(pr, *steps[i + 1]))
                    ps = emit_qk(*steps[i + 1])
                total = LAST_BUDGET if pr == NPAIR - 1 else TOTAL_BUDGET
                spent = drain_dq_ns(DQ_QUOTA)
                drain_eq_ns(total - spent)
        drain_to(10**9)
        drain_eq_ns(10**9)

    nc.compile()
    return nc


def _get_nc():
    if "nc" not in _CACHE:
        _CACHE["nc"] = _build_bass()
    return _CACHE["nc"]


def _host_consts():
    if "consts" not in _CACHE:
        qq = np.arange(P)[None, :]
        kk = np.arange(P)[:, None]
        _CACHE["consts"] = {
            "cm": (qq >= kk).astype(_bf16),
        }
    return _CACHE["consts"]


def make_in_maps(inputs):
    hs = np.asarray(inputs["hidden_states"], dtype=np.float32)
    am = np.asarray(inputs["attention_mask"], dtype=np.float32)
    Wq = np.asarray(inputs["Wq"], dtype=np.float32)
    bq = np.asarray(inputs["bq"], dtype=np.float32)
    Wk = np.asarray(inputs["Wk"], dtype=np.float32)
    bk = np.asarray(inputs["bk"], dtype=np.float32)
    Wv = np.asarray(inputs["Wv"], dtype=np.float32)
    bv = np.asarray(inputs["bv"], dtype=np.float32)

    consts = _host_consts()
    in_maps = []
    for c in range(NCORES):
        b, hg = c // 2, c % 2
        fsl = slice(hg * F, (hg + 1) * F)
        in_maps.append(
            {
                "xtb": np.ascontiguousarray(hs[b].T.astype(_bf16)),
                "wqtb": np.ascontiguousarray(Wq[fsl].T.astype(_bf16)),
                "wktb": np.ascontiguousarray(Wk[fsl].T.astype(_bf16)),
                "wvtb": np.ascontiguousarray(Wv[fsl].T.astype(_bf16)),
                "bqt": np.ascontiguousarray(bq[fsl].reshape(NPAIR, P).T),
                "bkt": np.ascontiguousarray(bk[fsl].reshape(NPAIR, P).T),
                "bvb": np.broadcast_to(bv[fsl], (P, F)).copy(),
                "maskb": np.ascontiguousarray((am[b, 0, 0] / 8.0).reshape(SCH, P).T),
                "cm": consts["cm"],
            }
        )
    return in_maps


def assemble_out(results):
    out = np.empty((B, S, H), dtype=np.float32)
    for c in range(NCORES):
        b, hg = c // 2, c % 2
        out[b, :, hg * F : (hg + 1) * F] = results[c]["out"]
    return out


def kernel(**inputs):
    from concourse.bass_utils import run_bass_kernel_spmd

    in_maps = make_in_maps(inputs)
    nc = _get_nc()
    res = run_bass_kernel_spmd(nc, in_maps, list(range(NCORES)))
    return assemble_out(res.results)


if __name__ == "__main__":
    rng = np.random.default_rng(0)
    ins = {
        "hidden_states": rng.standard_normal((B, S, H)).astype(np.float32),
        "attention_mask": np.zeros((B, 1, 1, S), np.float32),
        "Wq": (rng.standard_normal((H, H)) / 32.0).astype(np.float32),
        "bq": np.zeros(H, np.float32),
        "Wk": (rng.standard_normal((H, H)) / 32.0).astype(np.float32),
        "bk": np.zeros(H, np.float32),
        "Wv": (rng.standard_normal((H, H)) / 32.0).astype(np.float32),
        "bv": np.zeros(H, np.float32),
    }
    o = kernel(**ins)
    print("out", o.shape, o.dtype, float(np.abs(o).max()))


# revision 24
# speedup vs baseline: 1.1722x; 1.1722x over previous
"""Causal self-attention on 8 TRN2 NeuronCores.

Sharding: B=4 batches x 16 heads -> 64 (b,h) pairs; core c handles batch
b=c//2 and head-group hg=c%2 (8 heads = 512 of the 1024 features).
Q/K/V projection weights are row-sliced per head group (column-sharded in
the x @ W.T sense), so each core computes its own (b, 8-head) slice of the
S x S attention with no cross-core communication.

Kernel design:
- Matmuls contract over SBUF partitions, so X and the weight slices are
  shipped pre-transposed ([H, S] / [H, F]) in bf16; X^T / W^T tiles then
  load with plain (fast) DMA. Projections run bf16 x bf16 with fp32 PSUM
  accumulation; Q^T/K^T are rounded to bf16 (QK gets the FWL fast weight
  load), biases are added in fp32 during the PSUM->SBUF copyback.
- Scores are computed transposed, S^T[k, q] = (K^T)^T Q^T per 128-key
  chunk with a 512-wide q window; head parity picks partitions 0-63 vs
  64-127, whose K=64 matmuls run concurrently in separate PE row groups.
- No row-max subtraction: scaled scores are ~N(0,1), exp is safe in fp32.
  exp runs on ScalarE straight from PSUM with the attention-mask bias and
  the 1/sqrt(64) scale fused in. On diagonal tiles the q window of
  QK/exp/AV is narrowed; only one partial 128-col window per head needs a
  0/1 causal-mask multiply after exp.
- AV runs in natural layout: out[q, d+1] += (P^T chunk).T @ V_aug with a
  ones-column appended to V, so each accumulator's column HD is the
  softmax denominator; normalize = reciprocal + per-partition scalar mul,
  no output transposes. Full 128-row PE utilization.
- The kernel is ScalarE-bound: the exp stream (~160us of ACT) is the
  critical path. All other work (projections of later pairs, V chunks,
  AV/normalize/output) is drained from a fill queue BETWEEN QK steps,
  paced by estimated PE cost (~a QK period's worth per step) so the exp
  cadence stays tight, with explicit deadline markers forcing units
  needed by an upcoming QK step to drain first. This keeps TensorE dense
  (HAM clock stays at 2.4 GHz) without letting fill bursts starve exp.
"""

import sys

if "/opt/trn_rl_repo" not in sys.path:
    sys.path.insert(0, "/opt/trn_rl_repo")

import numpy as np
import ml_dtypes

_bf16 = np.dtype(ml_dtypes.bfloat16)

B, S, H, NH = 4, 2048, 1024, 16
HD = 64
NCORES = 8
F = 512  # features per core (8 heads)
NHEADS = 8  # heads per core
NPAIR = 4  # head pairs per core
HCH = H // 128  # 8 hidden chunks
SCH = S // 128  # 16 sequence chunks
P = 128

_CACHE = {}

# fill pacing: estimated PE-ns drained between consecutive QK steps.
# DQ_QUOTA bounds how much deadline (projection) work runs per step; the
# rest of TOTAL goes to elastic work (V chunks, AV/normalize/output).
# Keeping QK+fill under the ~1.1us exp period keeps the next QK punctual
# so the exp stream never data-waits; leftovers drain in the last pair's
# window (bigger budget there) while exp still covers them.
DQ_QUOTA = 450
TOTAL_BUDGET = 760
LAST_BUDGET = 1050


def _build_bass():
    import concourse.tile as tile
    from concourse import bacc, mybir
    from contextlib import ExitStack
    from collections import deque

    f32 = mybir.dt.float32
    EXP = mybir.ActivationFunctionType.Exp
    ADD = mybir.AluOpType.add
    MULT = mybir.AluOpType.mult

    nc = bacc.Bacc("TRN2", target_bir_lowering=False, debug=False, num_devices=NCORES)

    bf16 = mybir.dt.bfloat16
    x_d = nc.dram_tensor("xtb", [H, S], bf16, kind="ExternalInput").ap()
    wq_d = nc.dram_tensor("wqtb", [H, F], bf16, kind="ExternalInput").ap()
    wk_d = nc.dram_tensor("wktb", [H, F], bf16, kind="ExternalInput").ap()
    wv_d = nc.dram_tensor("wvtb", [H, F], bf16, kind="ExternalInput").ap()
    bqt_d = nc.dram_tensor("bqt", [P, NPAIR], f32, kind="ExternalInput").ap()
    bkt_d = nc.dram_tensor("bkt", [P, NPAIR], f32, kind="ExternalInput").ap()
    bvb_d = nc.dram_tensor("bvb", [P, F], f32, kind="ExternalInput").ap()
    maskb_d = nc.dram_tensor("maskb", [P, SCH], f32, kind="ExternalInput").ap()
    cm_d = nc.dram_tensor("cm", [P, P], bf16, kind="ExternalInput").ap()
    out_d = nc.dram_tensor("out", [S, F], f32, kind="ExternalOutput").ap()

    with tile.TileContext(nc) as tc, ExitStack() as ctx:
        const = ctx.enter_context(tc.tile_pool(name="const", bufs=1))
        xt_pool = ctx.enter_context(tc.tile_pool(name="xt", bufs=1))
        xt = xt_pool.tile([P, HCH, S], bf16, tag="xt")  # X^T via DMA transpose
        v_pool = ctx.enter_context(tc.tile_pool(name="v", bufs=1))
        v = v_pool.tile([P, SCH, NHEADS, HD + 1], bf16, tag="v")  # V + ones col

        # PSUM: QK pair slots (2 x 2 banks), small slots for projections and
        # V chunks (2 x 1 bank), AV accumulators (2 x 1 bank) = 8 banks.
        mmps = ctx.enter_context(tc.tile_pool(name="mmps", bufs=2, space="PSUM"))
        smps = ctx.enter_context(tc.tile_pool(name="smps", bufs=2, space="PSUM"))
        ops_ = ctx.enter_context(tc.tile_pool(name="ops", bufs=2, space="PSUM"))
        wt_pool = ctx.enter_context(tc.tile_pool(name="wt", bufs=3))
        qkt_pool = ctx.enter_context(tc.tile_pool(name="qkt", bufs=3))
        p_pool = ctx.enter_context(tc.tile_pool(name="pp", bufs=36))
        wtv_pool = ctx.enter_context(tc.tile_pool(name="wtv", bufs=1))
        obuf = ctx.enter_context(tc.tile_pool(name="obuf", bufs=8))
        rec_pool = ctx.enter_context(tc.tile_pool(name="rec", bufs=8))

        # ---- critical-path DMAs first: pair-0 weights + X^T block 0 (split
        # per hidden chunk so the first projection matmuls start after
        # ~160KB instead of 1.5MB), then the small consts. ----
        wtq0 = wt_pool.tile([P, HCH, P], bf16, tag="wtq")
        wtk0 = wt_pool.tile([P, HCH, P], bf16, tag="wtk")
        nc.sync.dma_start(wtq0[:], wq_d[:, 0:128].rearrange("(c p) f -> p c f", p=P))
        nc.sync.dma_start(wtk0[:], wk_d[:, 0:128].rearrange("(c p) f -> p c f", p=P))
        for j in range(HCH):
            nc.sync.dma_start(
                xt[:, j, 0:512],
                x_d[j * 128 : (j + 1) * 128, 0:512],
            )
        bqt = const.tile([P, NPAIR], f32, tag="bqt")
        nc.sync.dma_start(bqt[:], bqt_d[:])
        bkt = const.tile([P, NPAIR], f32, tag="bkt")
        nc.sync.dma_start(bkt[:], bkt_d[:])
        maskb = const.tile([P, SCH], f32, tag="maskb")
        nc.sync.dma_start(maskb[:], maskb_d[:])
        cm = const.tile([P, P], bf16, tag="cm")
        nc.sync.dma_start(cm[:], cm_d[:])
        bvb = const.tile([P, F], f32, tag="bvb")
        nc.sync.dma_start(bvb[:], bvb_d[:])
        nc.sync.dma_start(
            xt[:, :, 512:1024],
            x_d[:, 512:1024].rearrange("(c p) s -> p c s", p=P),
        )
        wtv = wtv_pool.tile([P, HCH, F], bf16, tag="wtv")
        nc.sync.dma_start(wtv[:], wv_d.rearrange("(c p) f -> p c f", p=P))
        for sb in (2, 3):
            nc.sync.dma_start(
                xt[:, :, sb * 512 : (sb + 1) * 512],
                x_d[:, sb * 512 : (sb + 1) * 512].rearrange("(c p) s -> p c s", p=P),
            )

        # V ones-column (softmax denominator trick): no input dependency
        nc.vector.memset(v[:, :, :, HD : HD + 1], 1.0)

        # ---- two fill queues: dq carries deadline work (W DMAs + Q/K
        # projections, tracked by markers so units land before the QK step
        # that reads them); eq carries elastic work (V chunks, AV units)
        # that may spill across pair windows without stalling exp. ----
        dq = deque()
        eq = deque()
        state = {"denq": 0, "dpop": 0}
        need = {}  # (pr, 'q'|'k', st) -> dq count required

        def push_d(cost, fn, marker=None):
            dq.append((cost, fn))
            state["denq"] += 1
            if marker is not None:
                need[marker] = state["denq"]

        def push_e(cost, fn):
            eq.append((cost, fn))

        def drain_to(n):
            while state["dpop"] < n and dq:
                _, fn = dq.popleft()
                state["dpop"] += 1
                fn()

        def drain_dq_ns(budget):
            acc = 0
            while dq and acc < budget:
                cost, fn = dq.popleft()
                state["dpop"] += 1
                fn()
                acc += cost
            return acc

        def drain_eq_ns(budget):
            acc = 0
            while eq and acc < budget:
                cost, fn = eq.popleft()
                fn()
                acc += cost

        def need_for(pr, qi, kc):
            r = 0
            m = need.get((pr, "q", qi))
            if m is not None:
                r = max(r, m)
            m = need.get((pr, "k", kc // 4))
            if m is not None:
                r = max(r, m)
            return r

        # ---- per head-pair: project Q^T/K^T; units are (cost, fn, marker)
        def make_pair_proj(pr, wt_tiles=None):
            if wt_tiles is None:
                wtq = wt_pool.tile([P, HCH, P], bf16, tag="wtq")
                wtk = wt_pool.tile([P, HCH, P], bf16, tag="wtk")
            else:
                wtq, wtk = wt_tiles
            qt = qkt_pool.tile([P, S], bf16, tag="qt")
            kt = qkt_pool.tile([P, S], bf16, tag="kt")
            units = []
            if wt_tiles is None:
                for wd, wt in ((wq_d, wtq), (wk_d, wtk)):

                    def dma_u(wt=wt, wd=wd):
                        nc.sync.dma_start(
                            wt[:],
                            wd[:, pr * 128 : (pr + 1) * 128].rearrange(
                                "(c p) f -> p c f", p=P
                            ),
                        )

                    units.append((100, dma_u, None))
            for st in range(4):
                for wt, dst, bias, tag in (
                    (wtq, qt, bqt, "q"),
                    (wtk, kt, bkt, "k"),
                ):
                    ps = smps.tile([P, F], f32, tag="sm")
                    for j0 in range(0, HCH, 2):

                        def mm_u(wt=wt, ps=ps, st=st, j0=j0):
                            for j in (j0, j0 + 1):
                                nc.tensor.matmul(
                                    ps[:],
                                    wt[:, j, :],
                                    xt[:, j, st * 512 : (st + 1) * 512],
                                    start=(j == 0),
                                    stop=(j == HCH - 1),
                                )

                        units.append((440, mm_u, None))

                    def cb_u(dst=dst, ps=ps, st=st, bias=bias):
                        nc.vector.tensor_scalar_add(
                            dst[:, st * 512 : (st + 1) * 512],
                            ps[:],
                            bias[:, pr : pr + 1],
                        )

                    units.append((80, cb_u, (pr, tag, st)))
            return qt, kt, units

        def v_unit(si, ha=0, hn=NHEADS):
            def u():
                w = hn * HD
                ps = smps.tile([P, F], f32, tag="sm")
                for j in range(HCH):
                    nc.tensor.matmul(
                        ps[:, 0:w],
                        xt[:, j, si * 128 : (si + 1) * 128],
                        wtv[:, j, ha * HD : ha * HD + w],
                        start=(j == 0),
                        stop=(j == HCH - 1),
                    )
                nc.vector.tensor_tensor(
                    v[:, si, ha : ha + hn, 0:HD],
                    ps[:, 0:w].rearrange("p (h d) -> p h d", h=hn),
                    bvb[:, ha * HD : ha * HD + w].rearrange("p (h d) -> p h d", h=hn),
                    ADD,
                )

            return u

        # ---- A0: pair-0 st0 projection runs inline (gates the first QK);
        # st1-st3 go into the deadline queue, the V chunks for heads 0-3
        # into the elastic queue (heads 4-5 / 6-7 are pushed at the pair-1
        # / pair-2 window starts — they are only read by later pairs' AV).
        pair_state = {0: make_pair_proj(0, wt_tiles=(wtq0, wtk0))}
        p0u = pair_state[0][2]
        assert len(p0u) == 40
        for _, fn, _m in p0u[0:10]:  # st0 inline
            fn()
        for c, fn, m in p0u[10:40]:  # st1-st3
            push_d(c, fn, m)
        for si in range(SCH):
            push_e(900, v_unit(si, 0, 4))

        def make_av_unit(pts, qc, h, hb, q0, qi):
            def av_unit():
                nkq = 4 * qi + qc + 1
                o_ps = ops_.tile([P, HD + 1], f32, tag="o")
                for kc in range(nkq):
                    nc.tensor.matmul(
                        o_ps[:],
                        pts[kc][:, hb + qc * 128 : hb + (qc + 1) * 128],
                        v[:, kc, h, :],
                        start=(kc == 0),
                        stop=(kc == nkq - 1),
                    )
                rec = rec_pool.tile([P, 1], f32, tag="rec")
                nc.vector.reciprocal(rec[:], o_ps[:, HD : HD + 1])
                otile = obuf.tile([P, HD], f32, tag="ob")
                nc.vector.tensor_scalar_mul(otile[:], o_ps[:, 0:HD], rec[:])
                nc.sync.dma_start(
                    out_d[q0 + qc * 128 : q0 + (qc + 1) * 128, h * HD : (h + 1) * HD],
                    otile[:],
                )

            return av_unit

        for pr in range(NPAIR):
            qt, kt = pair_state[pr][0], pair_state[pr][1]
            if pr > 0:
                for c, fn, m in pair_state[pr][2][22:42]:  # own st2/st3
                    push_d(c, fn, m)
            if pr == 1:
                for si in range(SCH):
                    push_e(500, v_unit(si, 4, 2))
            elif pr == 2:
                for si in range(SCH):
                    push_e(500, v_unit(si, 6, 2))
            if pr + 1 < NPAIR:
                pair_state[pr + 1] = make_pair_proj(pr + 1)
                for c, fn, m in pair_state[pr + 1][2][:22]:  # W DMA + st0/st1
                    push_d(c, fn, m)
            h0, h1 = 2 * pr, 2 * pr + 1

            def emit_qk(qi, kc, qt=qt, kt=kt):
                q0 = qi * 512
                off = kc - 4 * qi
                lo = off * 128 if off > 0 else 0
                ps = mmps.tile([P, 1024], f32, tag="mm")
                nc.tensor.matmul(
                    ps[:, lo:512],
                    kt[0:64, kc * 128 : (kc + 1) * 128],
                    qt[0:64, q0 + lo : q0 + 512],
                    start=True,
                    stop=True,
                    skip_group_check=True,
                )
                nc.tensor.matmul(
                    ps[:, 512 + lo : 1024],
                    kt[64:128, kc * 128 : (kc + 1) * 128],
                    qt[64:128, q0 + lo : q0 + 512],
                    start=True,
                    stop=True,
                    skip_group_check=True,
                )
                return ps

            steps = [(qi, kc) for qi in range(4) for kc in range(4 * (qi + 1))]
            pts_by_qi = {qi: [] for qi in range(4)}
            drain_to(need_for(pr, *steps[0]))
            ps = emit_qk(*steps[0])
            for i, (qi, kc) in enumerate(steps):
                q0 = qi * 512
                off = kc - 4 * qi
                lo = off * 128 if off > 0 else 0
                pt = p_pool.tile([P, 1024], bf16, tag="pt")
                pts_by_qi[qi].append(pt)
                if lo == 0:
                    nc.scalar.activation(
                        pt[:], ps[:], EXP, bias=maskb[:, kc : kc + 1], scale=0.125
                    )
                else:
                    nc.scalar.activation(
                        pt[:].rearrange("p (t q) -> p t q", t=2)[:, :, lo:512],
                        ps[:].rearrange("p (t q) -> p t q", t=2)[:, :, lo:512],
                        EXP,
                        bias=maskb[:, kc : kc + 1],
                        scale=0.125,
                    )
                if off >= 0:
                    pv = pt[:].rearrange("p (t q) -> p t q", t=2)[:, :, lo : lo + 128]
                    nc.vector.tensor_mul(
                        pv, pv, cm[:, None, :].to_broadcast((P, 2, P))
                    )
                    # all keys for q-chunk `off` of this q-tile are now in
                    # flight -> its AV units can be scheduled
                    nkq = 4 * qi + off + 1
                    for h, hb in ((h0, 0), (h1, 512)):
                        push_e(
                            35 * nkq + 250,
                            make_av_unit(pts_by_qi[qi], off, h, hb, q0, qi),
                        )
                if i + 1 < len(steps):
                    drain_to(need_for(pr, *steps[i + 1]))
                    ps = emit_qk(*steps[i + 1])
                total = LAST_BUDGET if pr == NPAIR - 1 else TOTAL_BUDGET
                spent = drain_dq_ns(DQ_QUOTA)
                drain_eq_ns(total - spent)
        drain_to(10**9)
        drain_eq_ns(10**9)

    nc.compile()
    return nc


def _get_nc():
    if "nc" not in _CACHE:
        _CACHE["nc"] = _build_bass()
    return _CACHE["nc"]


def _host_consts():
    if "consts" not in _CACHE:
        qq = np.arange(P)[None, :]
        kk = np.arange(P)[:, None]
        _CACHE["consts"] = {
            "cm": (qq >= kk).astype(_bf16),
        }
    return _CACHE["consts"]


def make_in_maps(inputs):
    hs = np.asarray(inputs["hidden_states"], dtype=np.float32)
    am = np.asarray(inputs["attention_mask"], dtype=np.float32)
    Wq = np.asarray(inputs["Wq"], dtype=np.float32)
    bq = np.asarray(inputs["bq"], dtype=np.float32)
    Wk = np.asarray(inputs["Wk"], dtype=np.float32)
    bk = np.asarray(inputs["bk"], dtype=np.float32)
    Wv = np.asarray(inputs["Wv"], dtype=np.float32)
    bv = np.asarray(inputs["bv"], dtype=np.float32)

    consts = _host_consts()
    in_maps = []
    for c in range(NCORES):
        b, hg = c // 2, c % 2
        fsl = slice(hg * F, (hg + 1) * F)
        in_maps.append(
            {
                "xtb": np.ascontiguousarray(hs[b].T.astype(_bf16)),
                "wqtb": np.ascontiguousarray(Wq[fsl].T.astype(_bf16)),
                "wktb": np.ascontiguousarray(Wk[fsl].T.astype(_bf16)),
                "wvtb": np.ascontiguousarray(Wv[fsl].T.astype(_bf16)),
                "bqt": np.ascontiguousarray(bq[fsl].reshape(NPAIR, P).T),
                "bkt": np.ascontiguousarray(bk[fsl].reshape(NPAIR, P).T),
                "bvb": np.broadcast_to(bv[fsl], (P, F)).copy(),
                "maskb": np.ascontiguousarray((am[b, 0, 0] / 8.0).reshape(SCH, P).T),
                "cm": consts["cm"],
            }
        )
    return in_maps


def assemble_out(results):
    out = np.empty((B, S, H), dtype=np.float32)
    for c in range(NCORES):
        b, hg = c // 2, c % 2
        out[b, :, hg * F : (hg + 1) * F] = results[c]["out"]
    return out


def kernel(**inputs):
    from concourse.bass_utils import run_bass_kernel_spmd

    in_maps = make_in_maps(inputs)
    nc = _get_nc()
    res = run_bass_kernel_spmd(nc, in_maps, list(range(NCORES)))
    return assemble_out(res.results)


if __name__ == "__main__":
    rng = np.random.default_rng(0)
    ins = {
        "hidden_states": rng.standard_normal((B, S, H)).astype(np.float32),
        "attention_mask": np.zeros((B, 1, 1, S), np.float32),
        "Wq": (rng.standard_normal((H, H)) / 32.0).astype(np.float32),
        "bq": np.zeros(H, np.float32),
        "Wk": (rng.standard_normal((H, H)) / 32.0).astype(np.float32),
        "bk": np.zeros(H, np.float32),
        "Wv": (rng.standard_normal((H, H)) / 32.0).astype(np.float32),
        "bv": np.zeros(H, np.float32),
    }
    o = kernel(**ins)
    print("out", o.shape, o.dtype, float(np.abs(o).max()))
